# revision 26
# baseline (speedup 1.0000x reference)
"""Trainium2 Bass kernel for nn_AttentionFusionHead (8-core data parallel).

Math (per batch row x_t [2048], x_g [1024]):
  t = Wt x_t + bt ; g = Wg x_g + bg                       (projections, C=2048)
  self-MHA over seq [t, g] (H=16 heads, D=128), softmax over 2 keys
  x_i = Wo_s o_i + bo_s ; ln_i = LayerNorm(x_i)
  cross-MHA: q from ln_i, k/v from [t, g]
  fused = 0.5 * sum_i (ln_i + Wo_c co_i + bo_c)
  out = Wout fused + bout                                  [1024]

Kernel strategy:
  - Pure data parallel: batch 8192 -> 1024 rows/core on 8 cores.
  - Feature-major layout on chip: activations stored transposed [features, batch]
    so every GEMM keeps weights stationary; no on-chip transposes. Host
    pre-transposes/tiles all weights and inputs.
  - Host weight folding: qkv/kv projections of the raw text/graph streams are
    composed with the input projections (e.g. Wqkv_s@Wt), removing the t/g
    materialization GEMMs and halving all graph-side contractions (K=1024).
  - softmax over 2 keys == sigmoid of score difference -> attention is
    elementwise ops + per-head partition reductions, done with an all-ones
    128x128 matmul that also broadcasts the result across partitions.
  - Attention fused into the qkv GEMMs per head (no qkv spill); cross-attn
    fused into the cross-q GEMMs; LayerNorm outputs stay SBUF-resident as the
    cross-q GEMM rhs (no ln spill round-trip).
  - Host folds 0.5*Wout@Wo_c so the cross output projection and the final
    projection collapse into one pair of accumulating chains per output tile.
  - DMA issue alternates between HWDGE (sync) and SWDGE (gpsimd) to double
    issue rate and queue parallelism; bf16 operands halve HBM traffic.

Measured on 8xNC trn2: ~1.63 ms HW exec, rel err ~5.2e-3 (bf16 mode).
f32r mode (KERNEL_MMDT=f32r) is higher precision but needs smaller pools;
the bf16 pool sizing is the tuned/tested configuration.
"""

import os
import sys

for _p in (
    "/root/.axon_site",
    "/root/.axon_site/_ro/trn_rl_repo",
    "/root/.axon_site/_ro/pypackages",
    "/opt/trn_rl_repo",
):
    if os.path.isdir(_p) and _p not in sys.path:
        sys.path.append(_p)

import numpy as np

import concourse.bacc as bacc
import concourse.mybir as mybir
import concourse.tile as tile
from concourse.bass_utils import run_bass_kernel_spmd

AF = mybir.ActivationFunctionType
F32 = mybir.dt.float32
F32R = mybir.dt.float32r
BF16 = mybir.dt.bfloat16

# Compute dtype for all matmuls / spilled activations.
# "bf16": half DMA traffic, ~3e-3 rel err.  "f32r": ~3e-4 rel err.
MMDT_NAME = os.environ.get("KERNEL_MMDT", "bf16")
MMDT = {"bf16": BF16, "f32r": F32R}[MMDT_NAME]

B = 8192
TEXT = 2048
GRAPH = 1024
C = 2048
OUT_DIM = 1024
H = 16
NCORES = 8
BC = B // NCORES  # batch per core
LN_EPS = 1e-5
ISQD = float(1.0 / np.sqrt(128.0))

_CACHE = {}


def _build(bc=BC):
    nch = bc // 512
    assert bc % 512 == 0
    nc = bacc.Bacc("TRN2", target_bir_lowering=False, debug=False)

    # ---- external inputs ----
    T = nc.dram_tensor("T", (TEXT, bc), MMDT, kind="ExternalInput")
    G = nc.dram_tensor("G", (GRAPH, bc), MMDT, kind="ExternalInput")

    def wdram(name, nk, nm):
        return nc.dram_tensor(name, (nm, 128, nk, 128), MMDT, kind="ExternalInput")

    Wqkvt = wdram("Wqkvt", 16, 48)
    Wqkvg = wdram("Wqkvg", 8, 48)
    Wos = wdram("Wos", 16, 16)
    Wqc = wdram("Wqc", 16, 16)
    Wkvt = wdram("Wkvt", 16, 32)
    Wkvg = wdram("Wkvg", 8, 32)
    WoutC = wdram("WoutC", 16, 8)   # 0.5 * Wout @ Wo_c
    WoutH = wdram("WoutH", 16, 8)   # 0.5 * Wout

    def bdram(name, nm):
        return nc.dram_tensor(name, (128, nm), F32, kind="ExternalInput")

    Bqkvt = bdram("Bqkvt", 48)
    Bqkvg = bdram("Bqkvg", 48)
    Bos = bdram("Bos", 16)
    Bqc = bdram("Bqc", 16)
    Bkvt = bdram("Bkvt", 32)
    Bkvg = bdram("Bkvg", 32)
    BoutP = bdram("BoutP", 8)       # bout + Wout @ bo_c
    LNW = bdram("LNW", 16)
    LNB = bdram("LNB", 16)
    ONES = nc.dram_tensor("ONES", (128, 128), MMDT, kind="ExternalInput")
    EPS = nc.dram_tensor("EPS", (128, 1), F32, kind="ExternalInput")

    OUT = nc.dram_tensor("OUT", (OUT_DIM, bc), F32, kind="ExternalOutput")

    # ---- internal DRAM spills (feature-major [features, batch]) ----
    o1d = nc.dram_tensor("o1d", (C, bc), MMDT)
    o2d = nc.dram_tensor("o2d", (C, bc), MMDT)
    x1d = nc.dram_tensor("x1d", (C, bc), MMDT)
    x2d = nc.dram_tensor("x2d", (C, bc), MMDT)
    ckv1d = nc.dram_tensor("ckv1d", (2 * C, bc), MMDT)
    ckv2d = nc.dram_tensor("ckv2d", (2 * C, bc), MMDT)
    lnsumd = nc.dram_tensor("lnsumd", (C, bc), MMDT)

    with tile.TileContext(nc) as tc:
        from contextlib import ExitStack

        with ExitStack() as ctx:
            big = MMDT_NAME == "bf16"
            rhs_pool = ctx.enter_context(tc.tile_pool(name="rhs", bufs=6 if big else 3))
            w_pool = ctx.enter_context(tc.tile_pool(name="w", bufs=3))
            eo_pool = ctx.enter_context(tc.tile_pool(name="eo", bufs=4))
            eo32_pool = ctx.enter_context(tc.tile_pool(name="eo32", bufs=3))
            qv_pool = ctx.enter_context(tc.tile_pool(name="qv", bufs=10 if big else 8))
            at_pool = ctx.enter_context(tc.tile_pool(name="at", bufs=16 if big else 12))
            st_pool = ctx.enter_context(tc.tile_pool(name="st", bufs=3))
            sq_pool = ctx.enter_context(tc.tile_pool(name="sqp", bufs=16))
            mu_pool = ctx.enter_context(tc.tile_pool(name="mu", bufs=2))
            cst_pool = ctx.enter_context(tc.tile_pool(name="cst", bufs=1))
            gps = ctx.enter_context(tc.tile_pool(name="gps", bufs=4, space="PSUM"))
            sps = ctx.enter_context(tc.tile_pool(name="sps", bufs=4, space="PSUM"))

            # round-robin DMA issue across the two DGE paths
            _dmac = [0]

            def dma(dst, src):
                _dmac[0] += 1
                eng = nc.sync if _dmac[0] % 2 else nc.gpsimd
                eng.dma_start(dst, src)

            ones = cst_pool.tile([128, 128], MMDT, tag="ones")
            nc.sync.dma_start(ones[:], ONES[:])
            eps = cst_pool.tile([128, 1], F32, tag="eps")
            nc.sync.dma_start(eps[:], EPS[:])

            def bias_tile(drt, nm):
                t = cst_pool.tile([128, nm], F32, tag=f"b_{drt.name}")
                nc.sync.dma_start(t[:], drt[:])
                return t

            def load_rhs(src, nk):
                """[nk*128, bc] feature-major DRAM -> half-K SBUF tiles."""
                view = src.ap().rearrange("(k p) n -> p k n", p=128)
                tiles = []
                for k0 in range(0, nk, 8):
                    kw = min(8, nk - k0)
                    t = rhs_pool.tile([128, 8, bc], MMDT, tag="rhs")
                    step = 1 if k0 == 0 else 2
                    for ks in range(0, kw, step):
                        ke = min(ks + step, kw)
                        if k0 == 0 and ks == 0:
                            h2 = bc // 2
                            dma(t[:, 0:1, 0:h2], view[:, 0:1, 0:h2])
                            dma(t[:, 0:1, h2:bc], view[:, 0:1, h2:bc])
                        else:
                            dma(t[:, ks:ke, :], view[:, k0 + ks : k0 + ke, :])
                    tiles.append(t)
                return tiles

            def rhs_slice(tiles, k, ch):
                return tiles[k // 8][:, k % 8, ch * 512 : (ch + 1) * 512]

            _wn = [0]

            def load_w(wd, m, nk):
                _wn[0] += 1
                w = w_pool.tile([128, 16, 128], MMDT, tag="w", name=f"wt{_wn[0]}")
                for k0 in range(0, nk, 4):
                    k1 = min(k0 + 4, nk)
                    dma(w[:, k0:k1, :], wd.ap()[m, :, k0:k1, :])
                return w

            def mm_chain(acc, w, rhs, nk, ch):
                for k in range(nk):
                    nc.tensor.matmul(
                        acc[:],
                        w[:, k, :],
                        rhs_slice(rhs, k, ch),
                        start=(k == 0),
                        stop=(k == nk - 1),
                    )

            def gemm(wd, nk, nm, rhs, out_cb, name):
                with nc.named_scope(name):
                    for m in range(nm):
                        w = load_w(wd, m, nk)
                        for ch in range(nch):
                            acc = gps.tile([128, 512], F32, tag="gacc")
                            mm_chain(acc, w, rhs, nk, ch)
                            out_cb(m, ch, acc)

            def spill_epilogue(dst, btile):
                def cb(m, ch, acc):
                    o = eo_pool.tile([128, 512], MMDT, tag="eo")
                    nc.scalar.activation(
                        o[:], acc[:], AF.Identity, bias=btile[:, m : m + 1]
                    )
                    dma(
                        dst.ap()[m * 128 : (m + 1) * 128, ch * 512 : (ch + 1) * 512],
                        o[:],
                    )

                return cb

            _tn = [0]

            def wide(pool, dt, tag):
                _tn[0] += 1
                return pool.tile([128, bc], dt, tag=tag, name=f"wt_{tag}{_tn[0]}")

            def half(pool, dt, tag):
                _tn[0] += 1
                return pool.tile([128, 512], dt, tag=tag, name=f"ht_{tag}{_tn[0]}")

            def qkv_head(wd, bt, nk, rhs, m, name_unused, w=None):
                """One [128, bc] output stripe of a qkv projection."""
                o = wide(qv_pool, MMDT, "qv")
                if w is None:
                    w = load_w(wd, m, nk)
                for ch in range(nch):
                    acc = gps.tile([128, 512], F32, tag="gacc")
                    mm_chain(acc, w, rhs, nk, ch)
                    nc.scalar.activation(
                        o[:, ch * 512 : (ch + 1) * 512],
                        acc[:],
                        AF.Identity,
                        bias=bt[:, m : m + 1],
                    )
                return o

            def attn_core(q1, q2, k1, k2, v1, v2, sum_mode, out_slice=None):
                """Per-head attention over 2 keys on wide [128, bc] tiles.

                sum_mode=False -> returns (o1, o2); True -> returns co1+co2."""
                dk = wide(at_pool, MMDT, "at")
                nc.vector.tensor_sub(dk[:], k1[:], k2[:])
                dv = wide(at_pool, MMDT, "at")
                nc.vector.tensor_sub(dv[:], v1[:], v2[:])
                p1 = wide(at_pool, MMDT, "at")
                nc.vector.tensor_mul(p1[:], q1[:], dk[:])
                p2 = wide(at_pool, MMDT, "at")
                nc.vector.tensor_mul(p2[:], q2[:], dk[:])
                a1 = wide(at_pool, MMDT, "at")
                a2 = wide(at_pool, MMDT, "at")
                for ch in range(nch):
                    cs = slice(ch * 512, (ch + 1) * 512)
                    s1 = sps.tile([128, 512], F32, tag="sc")
                    nc.tensor.matmul(s1[:], ones[:], p1[:, cs])
                    s2 = sps.tile([128, 512], F32, tag="sc")
                    nc.tensor.matmul(s2[:], ones[:], p2[:, cs])
                    nc.scalar.activation(a1[:, cs], s1[:], AF.Sigmoid, scale=ISQD)
                    nc.scalar.activation(a2[:, cs], s2[:], AF.Sigmoid, scale=ISQD)
                if not sum_mode:
                    outs = []
                    for a_i in (a1, a2):
                        t1 = wide(at_pool, MMDT, "at")
                        nc.vector.tensor_mul(t1[:], a_i[:], dv[:])
                        oh = wide(at_pool, MMDT, "at")
                        nc.vector.tensor_add(oh[:], t1[:], v2[:])
                        outs.append(oh)
                    return outs
                asum = wide(at_pool, MMDT, "at")
                nc.vector.tensor_add(asum[:], a1[:], a2[:])
                t1 = wide(at_pool, MMDT, "at")
                nc.vector.tensor_mul(t1[:], asum[:], dv[:])
                nc.vector.scalar_tensor_tensor(
                    out_slice, v2[:], 2.0, t1[:],
                    op0=mybir.AluOpType.mult, op1=mybir.AluOpType.add,
                )
                return None

            def spill_wide(dst, row, t):
                for ch in range(nch):
                    cs = slice(ch * 512, (ch + 1) * 512)
                    dma(dst.ap()[row * 128 : (row + 1) * 128, cs], t[:, cs])

            # ================= phases =================
            w_h0 = load_w(Wqkvt, 0, 16)
            rt = load_rhs(T, 16)
            rg = load_rhs(G, 8)
            bqkvt = bias_tile(Bqkvt, 48)
            bqkvg = bias_tile(Bqkvg, 48)
            bos = bias_tile(Bos, 16)
            bqc = bias_tile(Bqc, 16)
            bkvt = bias_tile(Bkvt, 32)
            bkvg = bias_tile(Bkvg, 32)
            bout = bias_tile(BoutP, 8)
            lnw = bias_tile(LNW, 16)
            lnb = bias_tile(LNB, 16)

            # fused self-qkv + self-attention, per head
            with nc.named_scope("selfattn"):
                for h in range(H):
                    q1 = qkv_head(Wqkvt, bqkvt, 16, rt, h, "q1", w=w_h0 if h == 0 else None)
                    k1 = qkv_head(Wqkvt, bqkvt, 16, rt, 16 + h, "k1")
                    v1 = qkv_head(Wqkvt, bqkvt, 16, rt, 32 + h, "v1")
                    q2 = qkv_head(Wqkvg, bqkvg, 8, rg, h, "q2")
                    k2 = qkv_head(Wqkvg, bqkvg, 8, rg, 16 + h, "k2")
                    v2 = qkv_head(Wqkvg, bqkvg, 8, rg, 32 + h, "v2")
                    o1, o2 = attn_core(q1, q2, k1, k2, v1, v2, sum_mode=False)
                    spill_wide(o1d, h, o1)
                    spill_wide(o2d, h, o2)

            # cross k/v from raw streams (folded weights)
            gemm(Wkvt, 16, 32, rt, spill_epilogue(ckv1d, bkvt), "ckvt")
            gemm(Wkvg, 8, 32, rg, spill_epilogue(ckv2d, bkvg), "ckvg")

            # self o-projection
            ro1 = load_rhs(o1d, 16)
            gemm(Wos, 16, 16, ro1, spill_epilogue(x1d, bos), "oproj1")
            ro2 = load_rhs(o2d, 16)
            gemm(Wos, 16, 16, ro2, spill_epilogue(x2d, bos), "oproj2")

            # LayerNorm over features (partition-dim stats via ones-matmul)
            def layernorm(xd, name, add_from=None, sum_to=None):
                view = xd.ap().rearrange("(k p) n -> p k n", p=128)
                out_tiles = [
                    rhs_pool.tile([128, 8, bc], MMDT, tag="rhs", name=f"lnr_{name}{i}")
                    for i in range(2)
                ]
                with nc.named_scope(name):
                    for ch in range(nch):
                        cs = slice(ch * 512, (ch + 1) * 512)
                        accS = sps.tile([128, 512], F32, tag="sc")
                        xks = []
                        sqs = []
                        for k in range(16):
                            xk = half(at_pool, MMDT, "at")
                            dma(xk[:], view[:, k, cs])
                            xks.append(xk)
                            nc.tensor.matmul(
                                accS[:], ones[:], xk[:],
                                start=(k == 0), stop=(k == 15),
                            )
                            sq = sq_pool.tile([128, 512], MMDT, tag="sq", name=f"sq_{name}{ch}_{k}")
                            nc.scalar.activation(sq[:], xk[:], AF.Square)
                            sqs.append(sq)
                        accQ = sps.tile([128, 512], F32, tag="sc")
                        for k in range(16):
                            nc.tensor.matmul(
                                accQ[:], ones[:], sqs[k][:],
                                start=(k == 0), stop=(k == 15),
                            )
                        mu = mu_pool.tile([128, 512], F32, tag="mu")
                        nc.scalar.mul(mu[:], accS[:], 1.0 / C)
                        ex2 = st_pool.tile([128, 512], F32, tag="st")
                        nc.scalar.mul(ex2[:], accQ[:], 1.0 / C)
                        mu2 = st_pool.tile([128, 512], F32, tag="st")
                        nc.vector.tensor_mul(mu2[:], mu[:], mu[:])
                        var = st_pool.tile([128, 512], F32, tag="st")
                        nc.vector.tensor_sub(var[:], ex2[:], mu2[:])
                        sd = st_pool.tile([128, 512], F32, tag="st")
                        nc.scalar.activation(sd[:], var[:], AF.Sqrt, bias=eps[:, 0:1])
                        rr = mu_pool.tile([128, 512], F32, tag="rr")
                        nc.vector.reciprocal(rr[:], sd[:])
                        for m in range(16):
                            xc = st_pool.tile([128, 512], F32, tag="xc")
                            nc.vector.tensor_sub(xc[:], xks[m][:], mu[:])
                            xn = st_pool.tile([128, 512], F32, tag="xc")
                            nc.vector.tensor_mul(xn[:], xc[:], rr[:])
                            lt = out_tiles[m // 8][:, m % 8, cs]
                            nc.scalar.activation(
                                lt, xn[:], AF.Identity,
                                bias=lnb[:, m : m + 1], scale=lnw[:, m : m + 1],
                            )
                            if add_from is not None:
                                lns = eo_pool.tile([128, 512], MMDT, tag="eo")
                                nc.vector.tensor_add(
                                    lns[:], lt, add_from[m // 8][:, m % 8, cs]
                                )
                                rows = slice(m * 128, (m + 1) * 128)
                                dma(sum_to.ap()[rows, cs], lns[:])
                return out_tiles

            rln1 = layernorm(x1d, "ln1")
            rln2 = layernorm(x2d, "ln2", add_from=rln1, sum_to=lnsumd)

            # fused cross-q GEMM + cross-attention (summed over query positions);
            # the summed output is written straight into resident rhs tiles for
            # the final projection (no DRAM round trip)
            rcs = [
                rhs_pool.tile([128, 8, bc], MMDT, tag="rhs", name=f"rcs{i}")
                for i in range(2)
            ]
            with nc.named_scope("crossattn"):
                for h in range(H):
                    q1 = qkv_head(Wqc, bqc, 16, rln1, h, "cq1")
                    q2 = qkv_head(Wqc, bqc, 16, rln2, h, "cq2")

                    def ldw(src, row):
                        t = wide(qv_pool, MMDT, "qv")
                        for ch in range(nch):
                            cs = slice(ch * 512, (ch + 1) * 512)
                            dma(t[:, cs], src.ap()[row * 128 : (row + 1) * 128, cs])
                        return t

                    k1 = ldw(ckv1d, h)
                    v1 = ldw(ckv1d, 16 + h)
                    k2 = ldw(ckv2d, h)
                    v2 = ldw(ckv2d, 16 + h)
                    attn_core(q1, q2, k1, k2, v1, v2, sum_mode=True,
                              out_slice=rcs[h // 8][:, h % 8, :])

            # final projection: out = 0.5*Wout@Wo_c @ cosum + 0.5*Wout @ lnsum + b'
            rlns = load_rhs(lnsumd, 16)
            with nc.named_scope("outproj"):
                for m in range(8):
                    w1 = load_w(WoutC, m, 16)
                    w2 = load_w(WoutH, m, 16)
                    for ch in range(nch):
                        acc = gps.tile([128, 512], F32, tag="gacc")
                        for k in range(16):
                            nc.tensor.matmul(
                                acc[:], w1[:, k, :], rhs_slice(rcs, k, ch),
                                start=(k == 0), stop=False,
                            )
                        for k in range(16):
                            nc.tensor.matmul(
                                acc[:], w2[:, k, :], rhs_slice(rlns, k, ch),
                                start=False, stop=(k == 15),
                            )
                        ot = eo32_pool.tile([128, 512], F32, tag="eo32", name=f"ot{m}_{ch}")
                        nc.scalar.activation(
                            ot[:], acc[:], AF.Identity, bias=bout[:, m : m + 1]
                        )
                        dma(
                            OUT.ap()[m * 128 : (m + 1) * 128, ch * 512 : (ch + 1) * 512],
                            ot[:],
                        )

    nc.compile()
    return nc


def _to_mmdt(a):
    if MMDT_NAME == "bf16":
        import ml_dtypes

        return np.asarray(a, dtype=ml_dtypes.bfloat16)
    return np.asarray(a, dtype=np.float32)


def _pack_w(Wmat):
    """[M_out, K_in] torch-style weight -> stationary lhsT tiles [nm,128,nk,128]."""
    M, K = Wmat.shape
    nm, nk = M // 128, K // 128
    W4 = Wmat.reshape(nm, 128, nk, 128)  # [m, mf, k, kp]
    return _to_mmdt(np.ascontiguousarray(W4.transpose(0, 3, 2, 1)))


def _pack_b(b):
    return np.ascontiguousarray(b.reshape(-1, 128).T, dtype=np.float32)


def _prep_inputs(
    text_embeddings, graph_embeddings, Wt, bt, Wg, bg,
    Wqkv_s, bqkv_s, Wo_s, bo_s, Wqkv_c, bqkv_c, Wo_c, bo_c,
    ln_w, ln_b, Wout, bout,
):
    f = np.float32
    Wt, bt, Wg, bg = (np.asarray(x, f) for x in (Wt, bt, Wg, bg))
    Wqkv_s, bqkv_s = np.asarray(Wqkv_s, f), np.asarray(bqkv_s, f)
    Wqkv_c, bqkv_c = np.asarray(Wqkv_c, f), np.asarray(bqkv_c, f)

    W2t = Wqkv_s @ Wt
    b2t = Wqkv_s @ bt + bqkv_s
    W2g = Wqkv_s @ Wg
    b2g = Wqkv_s @ bg + bqkv_s
    Wkv = Wqkv_c[C:]
    bkv = bqkv_c[C:]
    WKVt = Wkv @ Wt
    bKVt = Wkv @ bt + bkv
    WKVg = Wkv @ Wg
    bKVg = Wkv @ bg + bkv

    common = {
        "Wqkvt": _pack_w(W2t),
        "Wqkvg": _pack_w(W2g),
        "Wos": _pack_w(np.asarray(Wo_s, f)),
        "Wqc": _pack_w(Wqkv_c[:C]),
        "Wkvt": _pack_w(WKVt),
        "Wkvg": _pack_w(WKVg),
        "WoutC": _pack_w(0.5 * (np.asarray(Wout, f) @ np.asarray(Wo_c, f))),
        "WoutH": _pack_w(0.5 * np.asarray(Wout, f)),
        "Bqkvt": _pack_b(b2t),
        "Bqkvg": _pack_b(b2g),
        "Bos": _pack_b(np.asarray(bo_s, f)),
        "Bqc": _pack_b(bqkv_c[:C]),
        "Bkvt": _pack_b(bKVt),
        "Bkvg": _pack_b(bKVg),
        "BoutP": _pack_b(np.asarray(bout, f) + np.asarray(Wout, f) @ np.asarray(bo_c, f)),
        "LNW": _pack_b(np.asarray(ln_w, f)),
        "LNB": _pack_b(np.asarray(ln_b, f)),
        "ONES": _to_mmdt(np.ones((128, 128), f)),
        "EPS": np.full((128, 1), LN_EPS, f),
    }
    Tall = _to_mmdt(np.asarray(text_embeddings, f).T)
    Gall = _to_mmdt(np.asarray(graph_embeddings, f).T)
    in_maps = []
    for c in range(NCORES):
        m = dict(common)
        m["T"] = np.ascontiguousarray(Tall[:, c * BC : (c + 1) * BC])
        m["G"] = np.ascontiguousarray(Gall[:, c * BC : (c + 1) * BC])
        in_maps.append(m)
    return in_maps


def _run(in_maps, trace=False):
    if "nc" not in _CACHE:
        _CACHE["nc"] = _build()
    nc = _CACHE["nc"]
    kwargs = {}
    if trace:
        kwargs["trace"] = True
    res = run_bass_kernel_spmd(nc, in_maps, core_ids=list(range(NCORES)), **kwargs)
    out = np.concatenate(
        [np.asarray(res.results[c]["OUT"]).T for c in range(NCORES)], axis=0
    )
    return np.ascontiguousarray(out, dtype=np.float32), res


def kernel(**inputs):
    in_maps = _prep_inputs(**inputs)
    out, _ = _run(in_maps, trace=False)
    return out


def kernel_traced(**inputs):
    """Returns (output, BassKernelResults with exec_time_ns) — for test.py."""
    try:
        import ntff_shim  # noqa: F401
    except Exception:
        pass
    in_maps = _prep_inputs(**inputs)
    return _run(in_maps, trace=True)


# revision 28
# speedup vs baseline: 1.0153x; 1.0153x over previous
"""Trainium2 Bass kernel for nn_AttentionFusionHead (8-core data parallel).

Math (per batch row x_t [2048], x_g [1024]):
  t = Wt x_t + bt ; g = Wg x_g + bg                       (projections, C=2048)
  self-MHA over seq [t, g] (H=16 heads, D=128), softmax over 2 keys
  x_i = Wo_s o_i + bo_s ; ln_i = LayerNorm(x_i)
  cross-MHA: q from ln_i, k/v from [t, g]
  fused = 0.5 * sum_i (ln_i + Wo_c co_i + bo_c)
  out = Wout fused + bout                                  [1024]

Kernel strategy:
  - Pure data parallel: batch 8192 -> 1024 rows/core on 8 cores.
  - Feature-major layout on chip: activations stored transposed [features, batch]
    so every GEMM keeps weights stationary; no on-chip transposes. Host
    pre-transposes/tiles all weights and inputs.
  - Host weight folding: qkv/kv projections of the raw text/graph streams are
    composed with the input projections (e.g. Wqkv_s@Wt), removing the t/g
    materialization GEMMs and halving all graph-side contractions (K=1024).
  - softmax over 2 keys == sigmoid of score difference -> attention is
    elementwise ops + per-head partition reductions, done with an all-ones
    128x128 matmul that also broadcasts the result across partitions.
  - Attention fused into the qkv GEMMs per head (no qkv spill); cross-attn
    fused into the cross-q GEMMs; LayerNorm outputs stay SBUF-resident as the
    cross-q GEMM rhs (no ln spill round-trip).
  - Host folds 0.5*Wout@Wo_c so the cross output projection and the final
    projection collapse into one pair of accumulating chains per output tile.
  - DMA issue alternates between HWDGE (sync) and SWDGE (gpsimd) to double
    issue rate and queue parallelism; bf16 operands halve HBM traffic.

Measured on 8xNC trn2: ~1.60-1.62 ms HW exec, rel err ~5.2e-3 (bf16 mode).
f32r mode (KERNEL_MMDT=f32r) is higher precision but needs smaller pools;
the bf16 pool sizing is the tuned/tested configuration.
"""

import os
import sys

for _p in (
    "/root/.axon_site",
    "/root/.axon_site/_ro/trn_rl_repo",
    "/root/.axon_site/_ro/pypackages",
    "/opt/trn_rl_repo",
):
    if os.path.isdir(_p) and _p not in sys.path:
        sys.path.append(_p)

import numpy as np

import concourse.bacc as bacc
import concourse.mybir as mybir
import concourse.tile as tile
from concourse.bass_utils import run_bass_kernel_spmd

AF = mybir.ActivationFunctionType
F32 = mybir.dt.float32
F32R = mybir.dt.float32r
BF16 = mybir.dt.bfloat16

# Compute dtype for all matmuls / spilled activations.
# "bf16": half DMA traffic, ~3e-3 rel err.  "f32r": ~3e-4 rel err.
MMDT_NAME = os.environ.get("KERNEL_MMDT", "bf16")
MMDT = {"bf16": BF16, "f32r": F32R}[MMDT_NAME]

B = 8192
TEXT = 2048
GRAPH = 1024
C = 2048
OUT_DIM = 1024
H = 16
NCORES = 8
BC = B // NCORES  # batch per core
LN_EPS = 1e-5
ISQD = float(1.0 / np.sqrt(128.0))

_CACHE = {}


def _build(bc=BC):
    nch = bc // 512
    assert bc % 512 == 0
    nc = bacc.Bacc("TRN2", target_bir_lowering=False, debug=False)

    # ---- external inputs ----
    T = nc.dram_tensor("T", (TEXT, bc), MMDT, kind="ExternalInput")
    G = nc.dram_tensor("G", (GRAPH, bc), MMDT, kind="ExternalInput")

    def wdram(name, nk, nm):
        return nc.dram_tensor(name, (nm, 128, nk, 128), MMDT, kind="ExternalInput")

    Wqkvt = wdram("Wqkvt", 16, 48)
    Wqkvg = wdram("Wqkvg", 8, 48)
    Wos = wdram("Wos", 16, 16)
    Wqc = wdram("Wqc", 16, 16)
    Wkvt = wdram("Wkvt", 16, 32)
    Wkvg = wdram("Wkvg", 8, 32)
    WoutC = wdram("WoutC", 16, 8)   # 0.5 * Wout @ Wo_c
    WoutH = wdram("WoutH", 16, 8)   # 0.5 * Wout

    def bdram(name, nm):
        return nc.dram_tensor(name, (128, nm), F32, kind="ExternalInput")

    Bqkvt = bdram("Bqkvt", 48)
    Bqkvg = bdram("Bqkvg", 48)
    Bos = bdram("Bos", 16)
    Bqc = bdram("Bqc", 16)
    Bkvt = bdram("Bkvt", 32)
    Bkvg = bdram("Bkvg", 32)
    BoutP = bdram("BoutP", 8)       # bout + Wout @ bo_c
    LNW = bdram("LNW", 16)
    LNB = bdram("LNB", 16)
    ONES = nc.dram_tensor("ONES", (128, 128), MMDT, kind="ExternalInput")
    EPS = nc.dram_tensor("EPS", (128, 1), F32, kind="ExternalInput")

    OUT = nc.dram_tensor("OUT", (OUT_DIM, bc), F32, kind="ExternalOutput")

    # ---- internal DRAM spills (feature-major [features, batch]) ----
    o1d = nc.dram_tensor("o1d", (C, bc), MMDT)
    o2d = nc.dram_tensor("o2d", (C, bc), MMDT)
    x1d = nc.dram_tensor("x1d", (C, bc), MMDT)
    x2d = nc.dram_tensor("x2d", (C, bc), MMDT)
    ckv1d = nc.dram_tensor("ckv1d", (2 * C, bc), MMDT)
    ckv2d = nc.dram_tensor("ckv2d", (2 * C, bc), MMDT)
    lnsumd = nc.dram_tensor("lnsumd", (C, bc), MMDT)

    with tile.TileContext(nc) as tc:
        from contextlib import ExitStack

        with ExitStack() as ctx:
            big = MMDT_NAME == "bf16"
            rhs_pool = ctx.enter_context(tc.tile_pool(name="rhs", bufs=6 if big else 3))
            w_pool = ctx.enter_context(tc.tile_pool(name="w", bufs=3))
            eo_pool = ctx.enter_context(tc.tile_pool(name="eo", bufs=4))
            eo32_pool = ctx.enter_context(tc.tile_pool(name="eo32", bufs=3))
            qv_pool = ctx.enter_context(tc.tile_pool(name="qv", bufs=12 if big else 8))
            at_pool = ctx.enter_context(tc.tile_pool(name="at", bufs=16 if big else 12))
            st_pool = ctx.enter_context(tc.tile_pool(name="st", bufs=3))
            sq_pool = ctx.enter_context(tc.tile_pool(name="sqp", bufs=16))
            mu_pool = ctx.enter_context(tc.tile_pool(name="mu", bufs=1))
            cst_pool = ctx.enter_context(tc.tile_pool(name="cst", bufs=1))
            gps = ctx.enter_context(tc.tile_pool(name="gps", bufs=4, space="PSUM"))
            sps = ctx.enter_context(tc.tile_pool(name="sps", bufs=4, space="PSUM"))

            # round-robin DMA issue across the two DGE paths
            _dmac = [0]

            def dma(dst, src):
                _dmac[0] += 1
                eng = nc.sync if _dmac[0] % 2 else nc.gpsimd
                eng.dma_start(dst, src)

            ones = cst_pool.tile([128, 128], MMDT, tag="ones")
            nc.sync.dma_start(ones[:], ONES[:])
            eps = cst_pool.tile([128, 1], F32, tag="eps")
            nc.sync.dma_start(eps[:], EPS[:])

            def bias_tile(drt, nm):
                t = cst_pool.tile([128, nm], F32, tag=f"b_{drt.name}")
                nc.sync.dma_start(t[:], drt[:])
                return t

            def load_rhs(src, nk):
                """[nk*128, bc] feature-major DRAM -> half-K SBUF tiles."""
                view = src.ap().rearrange("(k p) n -> p k n", p=128)
                tiles = []
                for k0 in range(0, nk, 8):
                    kw = min(8, nk - k0)
                    t = rhs_pool.tile([128, 8, bc], MMDT, tag="rhs")
                    step = 1 if k0 == 0 else 2
                    for ks in range(0, kw, step):
                        ke = min(ks + step, kw)
                        if k0 == 0 and ks == 0:
                            h2 = bc // 2
                            dma(t[:, 0:1, 0:h2], view[:, 0:1, 0:h2])
                            dma(t[:, 0:1, h2:bc], view[:, 0:1, h2:bc])
                        else:
                            dma(t[:, ks:ke, :], view[:, k0 + ks : k0 + ke, :])
                    tiles.append(t)
                return tiles

            def rhs_slice(tiles, k, ch):
                return tiles[k // 8][:, k % 8, ch * 512 : (ch + 1) * 512]

            _wn = [0]

            def load_w(wd, m, nk):
                _wn[0] += 1
                w = w_pool.tile([128, 16, 128], MMDT, tag="w", name=f"wt{_wn[0]}")
                for k0 in range(0, nk, 4):
                    k1 = min(k0 + 4, nk)
                    dma(w[:, k0:k1, :], wd.ap()[m, :, k0:k1, :])
                return w

            def mm_chain(acc, w, rhs, nk, ch):
                for k in range(nk):
                    nc.tensor.matmul(
                        acc[:],
                        w[:, k, :],
                        rhs_slice(rhs, k, ch),
                        start=(k == 0),
                        stop=(k == nk - 1),
                    )

            def gemm(wd, nk, nm, rhs, out_cb, name):
                with nc.named_scope(name):
                    for m in range(nm):
                        w = load_w(wd, m, nk)
                        for ch in range(nch):
                            acc = gps.tile([128, 512], F32, tag="gacc")
                            mm_chain(acc, w, rhs, nk, ch)
                            out_cb(m, ch, acc)

            def spill_epilogue(dst, btile):
                def cb(m, ch, acc):
                    o = eo_pool.tile([128, 512], MMDT, tag="eo")
                    nc.scalar.activation(
                        o[:], acc[:], AF.Identity, bias=btile[:, m : m + 1]
                    )
                    dma(
                        dst.ap()[m * 128 : (m + 1) * 128, ch * 512 : (ch + 1) * 512],
                        o[:],
                    )

                return cb

            _tn = [0]

            def wide(pool, dt, tag):
                _tn[0] += 1
                return pool.tile([128, bc], dt, tag=tag, name=f"wt_{tag}{_tn[0]}")

            def half(pool, dt, tag):
                _tn[0] += 1
                return pool.tile([128, 512], dt, tag=tag, name=f"ht_{tag}{_tn[0]}")

            def qkv_head(wd, bt, nk, rhs, m, name_unused, w=None):
                """One [128, bc] output stripe of a qkv projection."""
                o = wide(qv_pool, MMDT, "qv")
                if w is None:
                    w = load_w(wd, m, nk)
                for ch in range(nch):
                    acc = gps.tile([128, 512], F32, tag="gacc")
                    mm_chain(acc, w, rhs, nk, ch)
                    nc.scalar.activation(
                        o[:, ch * 512 : (ch + 1) * 512],
                        acc[:],
                        AF.Identity,
                        bias=bt[:, m : m + 1],
                    )
                return o

            def attn_core(q1, q2, k1, k2, v1, v2, sum_mode, out_slice=None):
                """Per-head attention over 2 keys on wide [128, bc] tiles.

                sum_mode=False -> returns (o1, o2); True -> returns co1+co2."""
                dk = wide(at_pool, MMDT, "at")
                nc.vector.tensor_sub(dk[:], k1[:], k2[:])
                dv = wide(at_pool, MMDT, "at")
                nc.vector.tensor_sub(dv[:], v1[:], v2[:])
                p1 = wide(at_pool, MMDT, "at")
                nc.vector.tensor_mul(p1[:], q1[:], dk[:])
                p2 = wide(at_pool, MMDT, "at")
                nc.vector.tensor_mul(p2[:], q2[:], dk[:])
                a1 = wide(at_pool, MMDT, "at")
                a2 = wide(at_pool, MMDT, "at")
                for ch in range(nch):
                    cs = slice(ch * 512, (ch + 1) * 512)
                    s1 = sps.tile([128, 512], F32, tag="sc")
                    nc.tensor.matmul(s1[:], ones[:], p1[:, cs])
                    s2 = sps.tile([128, 512], F32, tag="sc")
                    nc.tensor.matmul(s2[:], ones[:], p2[:, cs])
                    nc.scalar.activation(a1[:, cs], s1[:], AF.Sigmoid, scale=ISQD)
                    nc.scalar.activation(a2[:, cs], s2[:], AF.Sigmoid, scale=ISQD)
                if not sum_mode:
                    outs = []
                    for a_i in (a1, a2):
                        t1 = wide(at_pool, MMDT, "at")
                        nc.vector.tensor_mul(t1[:], a_i[:], dv[:])
                        oh = wide(at_pool, MMDT, "at")
                        nc.vector.tensor_add(oh[:], t1[:], v2[:])
                        outs.append(oh)
                    return outs
                asum = wide(at_pool, MMDT, "at")
                nc.vector.tensor_add(asum[:], a1[:], a2[:])
                t1 = wide(at_pool, MMDT, "at")
                nc.vector.tensor_mul(t1[:], asum[:], dv[:])
                nc.vector.scalar_tensor_tensor(
                    out_slice, v2[:], 2.0, t1[:],
                    op0=mybir.AluOpType.mult, op1=mybir.AluOpType.add,
                )
                return None

            def spill_wide(dst, row, t):
                for ch in range(nch):
                    cs = slice(ch * 512, (ch + 1) * 512)
                    dma(dst.ap()[row * 128 : (row + 1) * 128, cs], t[:, cs])

            # ================= phases =================
            w_h0 = load_w(Wqkvt, 0, 16)
            rt = load_rhs(T, 16)
            rg = load_rhs(G, 8)
            bqkvt = bias_tile(Bqkvt, 48)
            bqkvg = bias_tile(Bqkvg, 48)
            bos = bias_tile(Bos, 16)
            bqc = bias_tile(Bqc, 16)
            bkvt = bias_tile(Bkvt, 32)
            bkvg = bias_tile(Bkvg, 32)
            bout = bias_tile(BoutP, 8)
            lnw = bias_tile(LNW, 16)
            lnb = bias_tile(LNB, 16)

            # fused self-qkv + self-attention, per head
            with nc.named_scope("selfattn"):
                for h in range(H):
                    q1 = qkv_head(Wqkvt, bqkvt, 16, rt, h, "q1", w=w_h0 if h == 0 else None)
                    k1 = qkv_head(Wqkvt, bqkvt, 16, rt, 16 + h, "k1")
                    v1 = qkv_head(Wqkvt, bqkvt, 16, rt, 32 + h, "v1")
                    q2 = qkv_head(Wqkvg, bqkvg, 8, rg, h, "q2")
                    k2 = qkv_head(Wqkvg, bqkvg, 8, rg, 16 + h, "k2")
                    v2 = qkv_head(Wqkvg, bqkvg, 8, rg, 32 + h, "v2")
                    o1, o2 = attn_core(q1, q2, k1, k2, v1, v2, sum_mode=False)
                    spill_wide(o1d, h, o1)
                    spill_wide(o2d, h, o2)

            # cross k/v from raw streams (folded weights)
            gemm(Wkvt, 16, 32, rt, spill_epilogue(ckv1d, bkvt), "ckvt")
            gemm(Wkvg, 8, 32, rg, spill_epilogue(ckv2d, bkvg), "ckvg")

            # self o-projection
            ro1 = load_rhs(o1d, 16)
            gemm(Wos, 16, 16, ro1, spill_epilogue(x1d, bos), "oproj1")
            ro2 = load_rhs(o2d, 16)
            gemm(Wos, 16, 16, ro2, spill_epilogue(x2d, bos), "oproj2")

            # LayerNorm over features (partition-dim stats via ones-matmul)
            def layernorm(xd, name, add_from=None, sum_to=None):
                view = xd.ap().rearrange("(k p) n -> p k n", p=128)
                out_tiles = [
                    rhs_pool.tile([128, 8, bc], MMDT, tag="rhs", name=f"lnr_{name}{i}")
                    for i in range(2)
                ]
                with nc.named_scope(name):
                    for ch in range(nch):
                        cs = slice(ch * 512, (ch + 1) * 512)
                        accS = sps.tile([128, 512], F32, tag="sc")
                        xks = []
                        sqs = []
                        for k in range(16):
                            xk = half(at_pool, MMDT, "at")
                            dma(xk[:], view[:, k, cs])
                            xks.append(xk)
                            nc.tensor.matmul(
                                accS[:], ones[:], xk[:],
                                start=(k == 0), stop=(k == 15),
                            )
                            sq = sq_pool.tile([128, 512], MMDT, tag="sq", name=f"sq_{name}{ch}_{k}")
                            nc.scalar.activation(sq[:], xk[:], AF.Square)
                            sqs.append(sq)
                        accQ = sps.tile([128, 512], F32, tag="sc")
                        for k in range(16):
                            nc.tensor.matmul(
                                accQ[:], ones[:], sqs[k][:],
                                start=(k == 0), stop=(k == 15),
                            )
                        mu = mu_pool.tile([128, 512], F32, tag="mu")
                        nc.scalar.mul(mu[:], accS[:], 1.0 / C)
                        ex2 = st_pool.tile([128, 512], F32, tag="st")
                        nc.scalar.mul(ex2[:], accQ[:], 1.0 / C)
                        mu2 = st_pool.tile([128, 512], F32, tag="st")
                        nc.vector.tensor_mul(mu2[:], mu[:], mu[:])
                        var = st_pool.tile([128, 512], F32, tag="st")
                        nc.vector.tensor_sub(var[:], ex2[:], mu2[:])
                        sd = st_pool.tile([128, 512], F32, tag="st")
                        nc.scalar.activation(sd[:], var[:], AF.Sqrt, bias=eps[:, 0:1])
                        rr = mu_pool.tile([128, 512], F32, tag="rr")
                        nc.vector.reciprocal(rr[:], sd[:])
                        for m in range(16):
                            xc = st_pool.tile([128, 512], F32, tag="xc")
                            nc.vector.tensor_sub(xc[:], xks[m][:], mu[:])
                            xn = st_pool.tile([128, 512], F32, tag="xc")
                            nc.vector.tensor_mul(xn[:], xc[:], rr[:])
                            lt = out_tiles[m // 8][:, m % 8, cs]
                            nc.scalar.activation(
                                lt, xn[:], AF.Identity,
                                bias=lnb[:, m : m + 1], scale=lnw[:, m : m + 1],
                            )
                            if add_from is not None:
                                lns = eo_pool.tile([128, 512], MMDT, tag="eo")
                                nc.vector.tensor_add(
                                    lns[:], lt, add_from[m // 8][:, m % 8, cs]
                                )
                                rows = slice(m * 128, (m + 1) * 128)
                                dma(sum_to.ap()[rows, cs], lns[:])
                return out_tiles

            rln1 = layernorm(x1d, "ln1")
            rln2 = layernorm(x2d, "ln2", add_from=rln1, sum_to=lnsumd)

            # fused cross-q GEMM + cross-attention (summed over query positions);
            # the summed output is written straight into resident rhs tiles for
            # the final projection (no DRAM round trip)
            rcs = [
                rhs_pool.tile([128, 8, bc], MMDT, tag="rhs", name=f"rcs{i}")
                for i in range(2)
            ]
            with nc.named_scope("crossattn"):
                for h in range(H):
                    q1 = qkv_head(Wqc, bqc, 16, rln1, h, "cq1")
                    q2 = qkv_head(Wqc, bqc, 16, rln2, h, "cq2")

                    def ldw(src, row):
                        t = wide(qv_pool, MMDT, "qv")
                        for ch in range(nch):
                            cs = slice(ch * 512, (ch + 1) * 512)
                            dma(t[:, cs], src.ap()[row * 128 : (row + 1) * 128, cs])
                        return t

                    k1 = ldw(ckv1d, h)
                    v1 = ldw(ckv1d, 16 + h)
                    k2 = ldw(ckv2d, h)
                    v2 = ldw(ckv2d, 16 + h)
                    attn_core(q1, q2, k1, k2, v1, v2, sum_mode=True,
                              out_slice=rcs[h // 8][:, h % 8, :])

            # final projection: out = 0.5*Wout@Wo_c @ cosum + 0.5*Wout @ lnsum + b'
            rlns = load_rhs(lnsumd, 16)
            with nc.named_scope("outproj"):
                for m in range(8):
                    w1 = load_w(WoutC, m, 16)
                    w2 = load_w(WoutH, m, 16)
                    for ch in range(nch):
                        acc = gps.tile([128, 512], F32, tag="gacc")
                        for k in range(16):
                            nc.tensor.matmul(
                                acc[:], w1[:, k, :], rhs_slice(rcs, k, ch),
                                start=(k == 0), stop=False,
                            )
                        for k in range(16):
                            nc.tensor.matmul(
                                acc[:], w2[:, k, :], rhs_slice(rlns, k, ch),
                                start=False, stop=(k == 15),
                            )
                        ot = eo32_pool.tile([128, 512], F32, tag="eo32", name=f"ot{m}_{ch}")
                        nc.scalar.activation(
                            ot[:], acc[:], AF.Identity, bias=bout[:, m : m + 1]
                        )
                        dma(
                            OUT.ap()[m * 128 : (m + 1) * 128, ch * 512 : (ch + 1) * 512],
                            ot[:],
                        )

    nc.compile()
    return nc


def _to_mmdt(a):
    if MMDT_NAME == "bf16":
        import ml_dtypes

        return np.asarray(a, dtype=ml_dtypes.bfloat16)
    return np.asarray(a, dtype=np.float32)


def _pack_w(Wmat):
    """[M_out, K_in] torch-style weight -> stationary lhsT tiles [nm,128,nk,128]."""
    M, K = Wmat.shape
    nm, nk = M // 128, K // 128
    W4 = Wmat.reshape(nm, 128, nk, 128)  # [m, mf, k, kp]
    return _to_mmdt(np.ascontiguousarray(W4.transpose(0, 3, 2, 1)))


def _pack_b(b):
    return np.ascontiguousarray(b.reshape(-1, 128).T, dtype=np.float32)


def _prep_inputs(
    text_embeddings, graph_embeddings, Wt, bt, Wg, bg,
    Wqkv_s, bqkv_s, Wo_s, bo_s, Wqkv_c, bqkv_c, Wo_c, bo_c,
    ln_w, ln_b, Wout, bout,
):
    f = np.float32
    Wt, bt, Wg, bg = (np.asarray(x, f) for x in (Wt, bt, Wg, bg))
    Wqkv_s, bqkv_s = np.asarray(Wqkv_s, f), np.asarray(bqkv_s, f)
    Wqkv_c, bqkv_c = np.asarray(Wqkv_c, f), np.asarray(bqkv_c, f)

    W2t = Wqkv_s @ Wt
    b2t = Wqkv_s @ bt + bqkv_s
    W2g = Wqkv_s @ Wg
    b2g = Wqkv_s @ bg + bqkv_s
    Wkv = Wqkv_c[C:]
    bkv = bqkv_c[C:]
    WKVt = Wkv @ Wt
    bKVt = Wkv @ bt + bkv
    WKVg = Wkv @ Wg
    bKVg = Wkv @ bg + bkv

    common = {
        "Wqkvt": _pack_w(W2t),
        "Wqkvg": _pack_w(W2g),
        "Wos": _pack_w(np.asarray(Wo_s, f)),
        "Wqc": _pack_w(Wqkv_c[:C]),
        "Wkvt": _pack_w(WKVt),
        "Wkvg": _pack_w(WKVg),
        "WoutC": _pack_w(0.5 * (np.asarray(Wout, f) @ np.asarray(Wo_c, f))),
        "WoutH": _pack_w(0.5 * np.asarray(Wout, f)),
        "Bqkvt": _pack_b(b2t),
        "Bqkvg": _pack_b(b2g),
        "Bos": _pack_b(np.asarray(bo_s, f)),
        "Bqc": _pack_b(bqkv_c[:C]),
        "Bkvt": _pack_b(bKVt),
        "Bkvg": _pack_b(bKVg),
        "BoutP": _pack_b(np.asarray(bout, f) + np.asarray(Wout, f) @ np.asarray(bo_c, f)),
        "LNW": _pack_b(np.asarray(ln_w, f)),
        "LNB": _pack_b(np.asarray(ln_b, f)),
        "ONES": _to_mmdt(np.ones((128, 128), f)),
        "EPS": np.full((128, 1), LN_EPS, f),
    }
    Tall = _to_mmdt(np.asarray(text_embeddings, f).T)
    Gall = _to_mmdt(np.asarray(graph_embeddings, f).T)
    in_maps = []
    for c in range(NCORES):
        m = dict(common)
        m["T"] = np.ascontiguousarray(Tall[:, c * BC : (c + 1) * BC])
        m["G"] = np.ascontiguousarray(Gall[:, c * BC : (c + 1) * BC])
        in_maps.append(m)
    return in_maps


def _run(in_maps, trace=False):
    if "nc" not in _CACHE:
        _CACHE["nc"] = _build()
    nc = _CACHE["nc"]
    kwargs = {}
    if trace:
        kwargs["trace"] = True
    res = run_bass_kernel_spmd(nc, in_maps, core_ids=list(range(NCORES)), **kwargs)
    out = np.concatenate(
        [np.asarray(res.results[c]["OUT"]).T for c in range(NCORES)], axis=0
    )
    return np.ascontiguousarray(out, dtype=np.float32), res


def kernel(**inputs):
    in_maps = _prep_inputs(**inputs)
    out, _ = _run(in_maps, trace=False)
    return out


def kernel_traced(**inputs):
    """Returns (output, BassKernelResults with exec_time_ns) — for test.py."""
    try:
        import ntff_shim  # noqa: F401
    except Exception:
        pass
    in_maps = _prep_inputs(**inputs)
    return _run(in_maps, trace=True)
